# revision 4
# baseline (speedup 1.0000x reference)
"""Trainium2 Bass kernel for nn_Attn_45423574123081 (sparse_attention).

Computes, for inputs enc [B=32, L=1024, D=64], W [64, 64], b [64]:
    energy = enc @ W.T + b                       # [B, L, D]
    scores[t, b, j] = energy[b, j] . enc[b, t]   # [T=1024, B, L]
    scores[t, :, t] = 0
    out = softmax(scores, axis=-1)

Sharding: data-parallel over batch; 4 batches per core on 8 NeuronCores.

Per-batch math on-core: S_b = E_b @ G_b^T with G_b = E_b @ W^T + b.

Precision strategy: all matmuls run in bf16 with an exact hi/lo split
(x = hi + lo, hi = bf16(x), lo = bf16(x - hi)).  The split operands are
stacked along the PE contraction axis (K <= 128 is free on the systolic
array), so S = E@G^T needs only two N=512 matmuls per PSUM bank:
    MM1 (K=128): [Ehi; Elo] @ [Glo; Ghi] = Ehi@Glo + Elo@Ghi   (crosses)
    MM2 (K=64):  Ehi @ Ghi                                      (main)
dropping only Elo@Glo (~1e-5 relative).  E^T is produced by PE
transposes of the bf16 hi|lo interleaved layout, 128 columns at a time.

Softmax skips the max-subtraction: |scores| <= ~50 for this problem
family and exp(50) is far below f32 overflow, so exp(s)/sum(exp(s)) is
numerically identical to the max-shifted form.  The diagonal zero is a
1-minus-identity mask multiply on the PSUM scores; exp runs on ScalarE
with a fused per-row accumulated sum; the normalize multiply runs on
GpSimd to keep VectorE off the critical path.
"""

import numpy as np

_B, _L, _D, _T = 32, 1024, 64, 1024
_N_CORES = 8
_BPC = _B // _N_CORES  # batches per core

_compiled_nc = None


def _build():
    global _compiled_nc
    if _compiled_nc is not None:
        return _compiled_nc

    import concourse.bacc as bacc
    import concourse.mybir as mybir
    from concourse import tile, masks

    dt = mybir.dt
    AF = mybir.ActivationFunctionType

    nc = bacc.Bacc(
        "TRN2",
        target_bir_lowering=False,
        debug=False,
        enable_asserts=False,
        num_devices=_N_CORES,
    )
    enc_d = nc.dram_tensor("enc", [_BPC, _L, _D], dt.float32, kind="ExternalInput")
    w_d = nc.dram_tensor("w", [_D, _D], dt.float32, kind="ExternalInput")
    b_d = nc.dram_tensor("bias", [_D], dt.float32, kind="ExternalInput")
    out_d = nc.dram_tensor("out", [_T, _BPC, _L], dt.float32, kind="ExternalOutput")

    with tile.TileContext(nc) as tc:
        with (
            tc.tile_pool(name="const", bufs=1) as cpool,
            tc.tile_pool(name="encp", bufs=2) as encpool,
            tc.tile_pool(name="enc2p", bufs=2) as enc2pool,
            tc.tile_pool(name="etp", bufs=2) as etpool,
            tc.tile_pool(name="gtp", bufs=2) as gtpool,
            tc.tile_pool(name="big", bufs=3) as bigpool,
            tc.tile_pool(name="small", bufs=4) as smallpool,
            tc.tile_pool(name="ps_s", bufs=2, space="PSUM") as ps_s_pool,
            tc.tile_pool(name="ps_tg", bufs=2, space="PSUM") as ps_tg_pool,
        ):
            ident_bf = cpool.tile([128, 128], dt.bfloat16)
            masks.make_identity(nc, ident_bf[:])
            ident_f = cpool.tile([_D, _D], dt.float32)
            masks.make_identity(nc, ident_f[:])
            # anti-identity: 1 everywhere except 0 on the diagonal
            anti = cpool.tile([128, 128], dt.float32)
            nc.gpsimd.memset(anti[:], 1.0)
            nc.gpsimd.affine_select(
                out=anti[:],
                in_=anti[:],
                compare_op=mybir.AluOpType.not_equal,
                fill=0.0,
                base=0,
                pattern=[[-1, 128]],
                channel_multiplier=1,
            )

            # --- W^T hi/lo: Wt = W^T (f32 PE transpose), then split.
            # w128 = [Wt_lo (p0-63); Wt_hi (p64-127)], wthi0 = Wt_hi at p0-63.
            w_sb = cpool.tile([_D, _D], dt.float32)
            nc.sync.dma_start(w_sb[:], w_d[:])
            ps_w = ps_tg_pool.tile([_D, _D], dt.float32, tag="ps_tg")
            nc.tensor.transpose(ps_w[:], w_sb[:], ident_f[:])
            wthi0 = cpool.tile([_D, _D], dt.bfloat16)
            nc.vector.tensor_copy(wthi0[:], ps_w[:])
            w128 = cpool.tile([128, _D], dt.bfloat16)
            nc.vector.tensor_sub(w128[: _D, :], ps_w[:], wthi0[:])
            nc.sync.dma_start(w128[_D :, :], wthi0[:])  # cross-partition move

            # --- bias hi/lo as [2, 64] K-stack + ones row for the rank-1 add
            b_row = cpool.tile([1, _D], dt.float32)
            nc.sync.dma_start(b_row[:], b_d[:].unsqueeze(0))
            b_hi = cpool.tile([1, _D], dt.bfloat16)
            nc.vector.tensor_copy(b_hi[:], b_row[:])
            b_lo = cpool.tile([1, _D], dt.bfloat16)
            nc.vector.tensor_sub(b_lo[:], b_row[:], b_hi[:])
            b2 = cpool.tile([2, _D], dt.bfloat16)
            nc.sync.dma_start(b2[0:1, :], b_hi[:])
            nc.sync.dma_start(b2[1:2, :], b_lo[:])
            ones2 = cpool.tile([2, 512], dt.bfloat16)
            nc.gpsimd.memset(ones2[:], 1.0)

            for bb in range(_BPC):
                # E natural layout: [128, (n=8, d=64)] f32, row l = n*128 + p
                enc_sb = encpool.tile([128, 8 * _D], dt.float32, tag="enc")
                nc.sync.dma_start(
                    enc_sb[:].rearrange("p (n d) -> p n d", n=8),
                    enc_d[bb].rearrange("(n p) d -> p n d", p=128),
                )
                # hi/lo split in natural layout: [128, (n=8, h=2, d=64)] bf16
                enc2 = enc2pool.tile([128, 2 * 8 * _D], dt.bfloat16, tag="enc2")
                enc2_v = enc2[:].rearrange("p (n h d) -> p n h d", n=8, h=2)
                enc_v = enc_sb[:].rearrange("p (n d) -> p n d", n=8)
                nc.vector.tensor_copy(enc2_v[:, :, 0, :], enc_v[:, :, :])
                nc.vector.tensor_sub(enc2_v[:, :, 1, :], enc_v[:, :, :], enc2_v[:, :, 0, :])
                # stacked transposes -> ET2 = [Ehi (p0-63); Elo (p64-127)] bf16
                ps_et2 = ps_tg_pool.tile([128, _L], dt.bfloat16, tag="ps_tg")
                for i in range(8):
                    nc.tensor.transpose(
                        ps_et2[:, i * 128 : (i + 1) * 128],
                        enc2[:, i * 128 : (i + 1) * 128],
                        ident_bf[:],
                    )
                et2 = etpool.tile([128, _L], dt.bfloat16, tag="et2")
                nc.vector.tensor_copy(et2[:], ps_et2[:])

                # --- G^T = W @ E^T + b in PSUM f32 [64, 1024]
                # per chunk: crosses (K=128) + main (K=64) + bias (K=2)
                ps_gt = ps_tg_pool.tile([_D, _L], dt.float32, tag="ps_tg")
                for c in range(2):
                    sl = slice(c * 512, (c + 1) * 512)
                    nc.tensor.matmul(
                        ps_gt[:, sl], w128[:], et2[:, sl], start=True, stop=False
                    )
                    nc.tensor.matmul(
                        ps_gt[:, sl], wthi0[:], et2[: _D, sl], start=False, stop=False
                    )
                    nc.tensor.matmul(
                        ps_gt[:, sl], b2[:], ones2[:], start=False, stop=True
                    )
                # split: gt_hi0 = Ghi at p0-63; gt2 = [Glo (p0-63); Ghi (p64-127)]
                gt_hi0 = gtpool.tile([_D, _L], dt.bfloat16, tag="gthi")
                nc.vector.tensor_copy(gt_hi0[:], ps_gt[:])
                gt2 = gtpool.tile([128, _L], dt.bfloat16, tag="gt2")
                nc.vector.tensor_sub(gt2[: _D, :], ps_gt[:], gt_hi0[:])
                nc.sync.dma_start(gt2[_D :, :], gt_hi0[:])  # cross-partition move

                # --- S row-blocks, two t-blocks per 1 MiB output DMA
                for i2 in range(4):
                    exp_sb = bigpool.tile([128, 2 * _L], dt.float32, tag="exp")
                    sums = smallpool.tile([128, 2], dt.float32, tag="sums")
                    recips = smallpool.tile([128, 2], dt.float32, tag="recips")
                    for h in range(2):
                        i = 2 * i2 + h
                        bsl = slice(i * 128, (i + 1) * 128)
                        ps_s = ps_s_pool.tile([128, _L], dt.float32, tag="ps_s")
                        for c in range(2):
                            sl = slice(c * 512, (c + 1) * 512)
                            # crosses: [Ehi;Elo] @ [Glo;Ghi]
                            nc.tensor.matmul(
                                ps_s[:, sl], et2[:, bsl], gt2[:, sl],
                                start=True, stop=False,
                            )
                            # main: Ehi @ Ghi
                            nc.tensor.matmul(
                                ps_s[:, sl], et2[: _D, bsl], gt_hi0[:, sl],
                                start=False, stop=True,
                            )
                        # zero the in-block diagonal (j == t)
                        nc.vector.tensor_mul(ps_s[:, bsl], ps_s[:, bsl], anti[:])
                        nc.scalar.activation(
                            exp_sb[:, h * _L : (h + 1) * _L],
                            ps_s[:],
                            AF.Exp,
                            accum_out=sums[:, h : h + 1],
                        )
                    nc.vector.reciprocal(recips[:], sums[:])
                    for h in range(2):
                        nc.gpsimd.tensor_scalar_mul(
                            exp_sb[:, h * _L : (h + 1) * _L],
                            exp_sb[:, h * _L : (h + 1) * _L],
                            recips[:, h : h + 1],
                        )
                    dst = (
                        out_d[2 * i2 * 128 : (2 * i2 + 2) * 128, bb : bb + 1, :]
                        .squeeze(1)
                        .rearrange("(h p) j -> p h j", p=128)
                    )
                    nc.sync.dma_start(dst, exp_sb[:].rearrange("p (h j) -> p h j", h=2))

    nc.compile()
    _compiled_nc = nc
    return nc


def _numpy_fallback(enc, W, b, tl):
    energy = np.einsum("bld,ed->ble", enc, W) + b
    scores = np.einsum("bjd,btd->tbj", energy, enc[:, :tl, :])
    t_idx = np.arange(tl)
    scores[t_idx, :, t_idx] = 0.0
    m = scores.max(axis=-1, keepdims=True)
    e = np.exp(scores - m)
    return (e / e.sum(axis=-1, keepdims=True)).astype(np.float32)


def _run(encoder_outputs, W, b, target_length=1024, **run_kwargs):
    enc = np.ascontiguousarray(np.asarray(encoder_outputs, dtype=np.float32))
    Wn = np.ascontiguousarray(np.asarray(W, dtype=np.float32))
    bn = np.ascontiguousarray(np.asarray(b, dtype=np.float32))
    tl = int(target_length)
    if enc.shape != (_B, _L, _D) or tl != _T:
        return _numpy_fallback(enc, Wn, bn, tl), None

    from concourse.bass_utils import run_bass_kernel_spmd

    nc = _build()
    in_maps = [
        {"enc": enc[i * _BPC : (i + 1) * _BPC], "w": Wn, "bias": bn}
        for i in range(_N_CORES)
    ]
    res = run_bass_kernel_spmd(nc, in_maps, list(range(_N_CORES)), **run_kwargs)
    out = np.concatenate(
        [res.results[i]["out"] for i in range(_N_CORES)], axis=1
    ).astype(np.float32)
    return out, res


def kernel(encoder_outputs, W, b, target_length=1024):
    out, _ = _run(encoder_outputs, W, b, target_length)
    return out


def kernel_profiled(encoder_outputs, W, b, target_length=1024):
    """Run with NTFF tracing; returns (output, BassKernelResults)."""
    return _run(encoder_outputs, W, b, target_length, trace=True)


# revision 6
# speedup vs baseline: 4.2541x; 4.2541x over previous
"""Trainium2 Bass kernel for nn_Attn_45423574123081 (sparse_attention).

Computes, for inputs enc [B=32, L=1024, D=64], W [64, 64], b [64]:
    energy = enc @ W.T + b                       # [B, L, D]
    scores[t, b, j] = energy[b, j] . enc[b, t]   # [T=1024, B, L]
    scores[t, :, t] = 0
    out = softmax(scores, axis=-1)

Sharding: data-parallel over batch; 4 batches per core on 8 NeuronCores.

Per-batch math on-core: S_b = E_b @ G_b^T with G_b = E_b @ W^T + b.

Precision strategy: all matmuls run in bf16 with an exact hi/lo split
(x = hi + lo, hi = bf16(x), lo = bf16(x - hi)).  The split operands are
stacked along the PE contraction axis (K <= 128 is free on the systolic
array), so S = E@G^T needs only two N=512 matmuls per PSUM bank:
    MM1 (K=128): [Ehi; Elo] @ [Glo; Ghi] = Ehi@Glo + Elo@Ghi   (crosses)
    MM2 (K=64):  Ehi @ Ghi                                      (main)
dropping only Elo@Glo (~1e-5 relative).  E^T is produced by PE
transposes of the bf16 hi|lo interleaved layout, 128 columns at a time.

Softmax skips the max-subtraction: |scores| <= ~50 for this problem
family and exp(50) is far below f32 overflow, so exp(s)/sum(exp(s)) is
numerically identical to the max-shifted form.  The diagonal zero is a
1-minus-identity mask multiply on the PSUM scores; exp runs on ScalarE
with a fused per-row accumulated sum; the normalize multiply runs on
GpSimd to keep VectorE off the critical path.
"""

import numpy as np

_B, _L, _D, _T = 32, 1024, 64, 1024
_N_CORES = 8
_BPC = _B // _N_CORES  # batches per core

_compiled_nc = None


def _build():
    global _compiled_nc
    if _compiled_nc is not None:
        return _compiled_nc

    import concourse.bacc as bacc
    import concourse.mybir as mybir
    from concourse import tile, masks

    dt = mybir.dt
    AF = mybir.ActivationFunctionType

    nc = bacc.Bacc(
        "TRN2",
        target_bir_lowering=False,
        debug=False,
        enable_asserts=False,
        num_devices=_N_CORES,
    )
    enc_d = nc.dram_tensor("enc", [_BPC, _L, _D], dt.float32, kind="ExternalInput")
    w_d = nc.dram_tensor("w", [_D, _D], dt.float32, kind="ExternalInput")
    b_d = nc.dram_tensor("bias", [_D], dt.float32, kind="ExternalInput")
    out_d = nc.dram_tensor("out", [_T, _BPC, _L], dt.float32, kind="ExternalOutput")

    with tile.TileContext(nc) as tc:
        with (
            tc.tile_pool(name="const", bufs=1) as cpool,
            tc.tile_pool(name="encp", bufs=2) as encpool,
            tc.tile_pool(name="enc2p", bufs=2) as enc2pool,
            tc.tile_pool(name="etp", bufs=2) as etpool,
            tc.tile_pool(name="gtp", bufs=2) as gtpool,
            tc.tile_pool(name="big", bufs=3) as bigpool,
            tc.tile_pool(name="small", bufs=4) as smallpool,
            tc.tile_pool(name="ps_s", bufs=2, space="PSUM") as ps_s_pool,
            tc.tile_pool(name="ps_tg", bufs=2, space="PSUM") as ps_tg_pool,
        ):
            ident_bf = cpool.tile([128, 128], dt.bfloat16)
            masks.make_identity(nc, ident_bf[:])
            ident_f = cpool.tile([_D, _D], dt.float32)
            masks.make_identity(nc, ident_f[:])
            # anti-identity: 1 everywhere except 0 on the diagonal
            anti = cpool.tile([128, 128], dt.float32)
            nc.gpsimd.memset(anti[:], 1.0)
            nc.gpsimd.affine_select(
                out=anti[:],
                in_=anti[:],
                compare_op=mybir.AluOpType.not_equal,
                fill=0.0,
                base=0,
                pattern=[[-1, 128]],
                channel_multiplier=1,
            )

            # --- W^T hi/lo: Wt = W^T (f32 PE transpose), then split.
            # w128 = [Wt_lo (p0-63); Wt_hi (p64-127)], wthi0 = Wt_hi at p0-63.
            w_sb = cpool.tile([_D, _D], dt.float32)
            nc.sync.dma_start(w_sb[:], w_d[:])
            ps_w = ps_tg_pool.tile([_D, _D], dt.float32, tag="ps_tg")
            nc.tensor.transpose(ps_w[:], w_sb[:], ident_f[:])
            wthi0 = cpool.tile([_D, _D], dt.bfloat16)
            nc.vector.tensor_copy(wthi0[:], ps_w[:])
            w128 = cpool.tile([128, _D], dt.bfloat16)
            nc.vector.tensor_sub(w128[: _D, :], ps_w[:], wthi0[:])
            nc.sync.dma_start(w128[_D :, :], wthi0[:])  # cross-partition move

            # --- bias hi/lo as [2, 64] K-stack + ones row for the rank-1 add
            b_row = cpool.tile([1, _D], dt.float32)
            nc.sync.dma_start(b_row[:], b_d[:].unsqueeze(0))
            b_hi = cpool.tile([1, _D], dt.bfloat16)
            nc.vector.tensor_copy(b_hi[:], b_row[:])
            b_lo = cpool.tile([1, _D], dt.bfloat16)
            nc.vector.tensor_sub(b_lo[:], b_row[:], b_hi[:])
            b2 = cpool.tile([2, _D], dt.bfloat16)
            nc.sync.dma_start(b2[0:1, :], b_hi[:])
            nc.sync.dma_start(b2[1:2, :], b_lo[:])
            ones2 = cpool.tile([2, 512], dt.bfloat16)
            nc.gpsimd.memset(ones2[:], 1.0)

            for bb in range(_BPC):
                # E natural layout: [128, (n=8, d=64)] f32, row l = n*128 + p
                enc_sb = encpool.tile([128, 8 * _D], dt.float32, tag="enc")
                nc.sync.dma_start(
                    enc_sb[:].rearrange("p (n d) -> p n d", n=8),
                    enc_d[bb].rearrange("(n p) d -> p n d", p=128),
                )
                # hi/lo split in natural layout: [128, (n=8, h=2, d=64)] bf16
                enc2 = enc2pool.tile([128, 2 * 8 * _D], dt.bfloat16, tag="enc2")
                enc2_v = enc2[:].rearrange("p (n h d) -> p n h d", n=8, h=2)
                enc_v = enc_sb[:].rearrange("p (n d) -> p n d", n=8)
                nc.vector.tensor_copy(enc2_v[:, :, 0, :], enc_v[:, :, :])
                nc.vector.tensor_sub(enc2_v[:, :, 1, :], enc_v[:, :, :], enc2_v[:, :, 0, :])
                # stacked transposes -> ET2 = [Ehi (p0-63); Elo (p64-127)] bf16
                ps_et2 = ps_tg_pool.tile([128, _L], dt.bfloat16, tag="ps_tg")
                for i in range(8):
                    nc.tensor.transpose(
                        ps_et2[:, i * 128 : (i + 1) * 128],
                        enc2[:, i * 128 : (i + 1) * 128],
                        ident_bf[:],
                    )
                et2 = etpool.tile([128, _L], dt.bfloat16, tag="et2")
                nc.vector.tensor_copy(et2[:], ps_et2[:])

                # --- G^T = W @ E^T + b in PSUM f32 [64, 1024]
                # per chunk: crosses (K=128) + main (K=64) + bias (K=2)
                ps_gt = ps_tg_pool.tile([_D, _L], dt.float32, tag="ps_tg")
                for c in range(2):
                    sl = slice(c * 512, (c + 1) * 512)
                    nc.tensor.matmul(
                        ps_gt[:, sl], w128[:], et2[:, sl], start=True, stop=False
                    )
                    nc.tensor.matmul(
                        ps_gt[:, sl], wthi0[:], et2[: _D, sl], start=False, stop=False
                    )
                    nc.tensor.matmul(
                        ps_gt[:, sl], b2[:], ones2[:], start=False, stop=True
                    )
                # split: gt_hi0 = Ghi at p0-63; gt2 = [Glo (p0-63); Ghi (p64-127)]
                gt_hi0 = gtpool.tile([_D, _L], dt.bfloat16, tag="gthi")
                nc.scalar.activation(gt_hi0[:], ps_gt[:], AF.Copy)
                gt2 = gtpool.tile([128, _L], dt.bfloat16, tag="gt2")
                nc.vector.tensor_sub(gt2[: _D, :], ps_gt[:], gt_hi0[:])
                nc.sync.dma_start(gt2[_D :, :], gt_hi0[:])  # cross-partition move

                # --- S row-blocks, two t-blocks per 1 MiB output DMA
                for i2 in range(4):
                    exp_sb = bigpool.tile([128, 2 * _L], dt.float32, tag="exp")
                    sums = smallpool.tile([128, 2], dt.float32, tag="sums")
                    recips = smallpool.tile([128, 2], dt.float32, tag="recips")
                    for h in range(2):
                        i = 2 * i2 + h
                        bsl = slice(i * 128, (i + 1) * 128)
                        ps_s = ps_s_pool.tile([128, _L], dt.float32, tag="ps_s")
                        for c in range(2):
                            sl = slice(c * 512, (c + 1) * 512)
                            # crosses: [Ehi;Elo] @ [Glo;Ghi]
                            nc.tensor.matmul(
                                ps_s[:, sl], et2[:, bsl], gt2[:, sl],
                                start=True, stop=False,
                            )
                            # main: Ehi @ Ghi
                            nc.tensor.matmul(
                                ps_s[:, sl], et2[: _D, bsl], gt_hi0[:, sl],
                                start=False, stop=True,
                            )
                        # zero the in-block diagonal (j == t)
                        nc.vector.tensor_mul(ps_s[:, bsl], ps_s[:, bsl], anti[:])
                        nc.scalar.activation(
                            exp_sb[:, h * _L : (h + 1) * _L],
                            ps_s[:],
                            AF.Exp,
                            accum_out=sums[:, h : h + 1],
                        )
                    nc.vector.reciprocal(recips[:], sums[:])
                    for h in range(2):
                        nc.vector.tensor_scalar_mul(
                            exp_sb[:, h * _L : (h + 1) * _L],
                            exp_sb[:, h * _L : (h + 1) * _L],
                            recips[:, h : h + 1],
                        )
                    dst = (
                        out_d[2 * i2 * 128 : (2 * i2 + 2) * 128, bb : bb + 1, :]
                        .squeeze(1)
                        .rearrange("(h p) j -> p h j", p=128)
                    )
                    nc.sync.dma_start(dst, exp_sb[:].rearrange("p (h j) -> p h j", h=2))

    nc.compile()
    _compiled_nc = nc
    return nc


def _numpy_fallback(enc, W, b, tl):
    energy = np.einsum("bld,ed->ble", enc, W) + b
    scores = np.einsum("bjd,btd->tbj", energy, enc[:, :tl, :])
    t_idx = np.arange(tl)
    scores[t_idx, :, t_idx] = 0.0
    m = scores.max(axis=-1, keepdims=True)
    e = np.exp(scores - m)
    return (e / e.sum(axis=-1, keepdims=True)).astype(np.float32)


def _run(encoder_outputs, W, b, target_length=1024, **run_kwargs):
    enc = np.ascontiguousarray(np.asarray(encoder_outputs, dtype=np.float32))
    Wn = np.ascontiguousarray(np.asarray(W, dtype=np.float32))
    bn = np.ascontiguousarray(np.asarray(b, dtype=np.float32))
    tl = int(target_length)
    if enc.shape != (_B, _L, _D) or tl != _T:
        return _numpy_fallback(enc, Wn, bn, tl), None

    from concourse.bass_utils import run_bass_kernel_spmd

    nc = _build()
    in_maps = [
        {"enc": enc[i * _BPC : (i + 1) * _BPC], "w": Wn, "bias": bn}
        for i in range(_N_CORES)
    ]
    res = run_bass_kernel_spmd(nc, in_maps, list(range(_N_CORES)), **run_kwargs)
    out = np.concatenate(
        [res.results[i]["out"] for i in range(_N_CORES)], axis=1
    ).astype(np.float32)
    return out, res


def kernel(encoder_outputs, W, b, target_length=1024):
    out, _ = _run(encoder_outputs, W, b, target_length)
    return out


def kernel_profiled(encoder_outputs, W, b, target_length=1024):
    """Run with NTFF tracing; returns (output, BassKernelResults)."""
    return _run(encoder_outputs, W, b, target_length, trace=True)


# revision 7
# speedup vs baseline: 4.8867x; 1.1487x over previous
"""Trainium2 Bass kernel for nn_Attn_45423574123081 (sparse_attention).

Computes, for inputs enc [B=32, L=1024, D=64], W [64, 64], b [64]:
    energy = enc @ W.T + b                       # [B, L, D]
    scores[t, b, j] = energy[b, j] . enc[b, t]   # [T=1024, B, L]
    scores[t, :, t] = 0
    out = softmax(scores, axis=-1)

Sharding: data-parallel over batch; 4 batches per core on 8 NeuronCores.

Per-batch math on-core: S_b = E_b @ G_b^T with G_b = E_b @ W^T + b.

Precision strategy: matmuls run in fp16.  E is cast once to fp16
(~5e-4 score error).  W, b and G use an exact fp16 hi/lo split
(x = hi + lo), with the split halves stacked along the PE contraction
axis — K <= 128 is free on the systolic array — so each 512-wide PSUM
bank of S needs exactly ONE K=128 matmul:
    [E16; E16] @ [Glo; Ghi] = E16 @ G
The E16 row duplication and the Ghi upper-half placement are done with
small SBUF->SBUF DMAs (cross-partition moves the compute engines can't
do).  G itself is computed as [Whi; Wlo] @ [E16; E16] + [bhi; blo] ones
rank-2 matmul, all operands exact except E16.

Softmax skips the max-subtraction: |scores| <= ~50 for this problem
family and exp(50) is far below f32 overflow.  The diagonal zero is a
1-minus-identity mask multiply on the PSUM scores; exp runs on ScalarE
with a fused per-row accumulated sum; normalize is a per-partition
scalar multiply on VectorE.
"""

import numpy as np

_B, _L, _D, _T = 32, 1024, 64, 1024
_N_CORES = 8
_BPC = _B // _N_CORES  # batches per core

_compiled_nc = None


def _build():
    global _compiled_nc
    if _compiled_nc is not None:
        return _compiled_nc

    import concourse.bacc as bacc
    import concourse.mybir as mybir
    from concourse import tile, masks

    dt = mybir.dt
    AF = mybir.ActivationFunctionType

    nc = bacc.Bacc(
        "TRN2",
        target_bir_lowering=False,
        debug=False,
        enable_asserts=False,
        num_devices=_N_CORES,
    )
    enc_d = nc.dram_tensor("enc", [_BPC, _L, _D], dt.float32, kind="ExternalInput")
    w_d = nc.dram_tensor("w", [_D, _D], dt.float32, kind="ExternalInput")
    b_d = nc.dram_tensor("bias", [_D], dt.float32, kind="ExternalInput")
    out_d = nc.dram_tensor("out", [_T, _BPC, _L], dt.float32, kind="ExternalOutput")

    with tile.TileContext(nc) as tc:
        with (
            tc.tile_pool(name="const", bufs=1) as cpool,
            tc.tile_pool(name="encp", bufs=2) as encpool,
            tc.tile_pool(name="enc16p", bufs=2) as enc16pool,
            tc.tile_pool(name="etp", bufs=2) as etpool,
            tc.tile_pool(name="gtp", bufs=2) as gtpool,
            tc.tile_pool(name="big", bufs=3) as bigpool,
            tc.tile_pool(name="small", bufs=4) as smallpool,
            tc.tile_pool(name="ps_s", bufs=2, space="PSUM") as ps_s_pool,
            tc.tile_pool(name="ps_et", bufs=2, space="PSUM") as ps_et_pool,
            tc.tile_pool(name="ps_gt", bufs=1, space="PSUM") as ps_gt_pool,
        ):
            ident_h = cpool.tile([128, 128], dt.float16)
            masks.make_identity(nc, ident_h[:])
            ident_f = cpool.tile([_D, _D], dt.float32)
            masks.make_identity(nc, ident_f[:])
            # anti-identity: 1 everywhere except 0 on the diagonal
            anti = cpool.tile([128, 128], dt.float32)
            nc.gpsimd.memset(anti[:], 1.0)
            nc.gpsimd.affine_select(
                out=anti[:],
                in_=anti[:],
                compare_op=mybir.AluOpType.not_equal,
                fill=0.0,
                base=0,
                pattern=[[-1, 128]],
                channel_multiplier=1,
            )

            # --- W^T hi/lo (exact fp16 split): w2 = [Whi (p0-63); Wlo (p64-127)]
            w_sb = cpool.tile([_D, _D], dt.float32)
            nc.sync.dma_start(w_sb[:], w_d[:])
            ps_w = ps_gt_pool.tile([_D, _D], dt.float32, tag="ps_gt")
            nc.tensor.transpose(ps_w[:], w_sb[:], ident_f[:])
            w_hi = cpool.tile([_D, _D], dt.float16)
            nc.vector.tensor_copy(w_hi[:], ps_w[:])
            w_lo = cpool.tile([_D, _D], dt.float16)
            nc.vector.tensor_sub(w_lo[:], ps_w[:], w_hi[:])
            w2 = cpool.tile([128, _D], dt.float16)
            nc.sync.dma_start(w2[: _D, :], w_hi[:])
            nc.sync.dma_start(w2[_D :, :], w_lo[:])

            # --- bias hi/lo as [2, 64] K-stack + ones rows for the rank-2 add
            b_row = cpool.tile([1, _D], dt.float32)
            nc.sync.dma_start(b_row[:], b_d[:].unsqueeze(0))
            b_hi = cpool.tile([1, _D], dt.float16)
            nc.vector.tensor_copy(b_hi[:], b_row[:])
            b_lo = cpool.tile([1, _D], dt.float16)
            nc.vector.tensor_sub(b_lo[:], b_row[:], b_hi[:])
            b2 = cpool.tile([2, _D], dt.float16)
            nc.sync.dma_start(b2[0:1, :], b_hi[:])
            nc.sync.dma_start(b2[1:2, :], b_lo[:])
            ones2 = cpool.tile([2, 512], dt.float16)
            nc.gpsimd.memset(ones2[:], 1.0)

            for bb in range(_BPC):
                # E natural layout: [128, (n=8, d=64)] f32, row l = n*128 + p
                enc_sb = encpool.tile([128, 8 * _D], dt.float32, tag="enc")
                nc.sync.dma_start(
                    enc_sb[:].rearrange("p (n d) -> p n d", n=8),
                    enc_d[bb].rearrange("(n p) d -> p n d", p=128),
                )
                enc16 = enc16pool.tile([128, 8 * _D], dt.float16, tag="enc16")
                nc.vector.tensor_copy(enc16[:], enc_sb[:])
                # E16^T via 8 PE transposes, then duplicate rows via SBUF DMA:
                # et16d = [E16^T (p0-63); E16^T (p64-127)]
                ps_et16 = ps_et_pool.tile([_D, _L], dt.float16, tag="ps_et")
                for i in range(8):
                    nc.tensor.transpose(
                        ps_et16[:, i * 128 : (i + 1) * 128],
                        enc16[:, i * _D : (i + 1) * _D],
                        ident_h[:],
                    )
                et16d = etpool.tile([128, _L], dt.float16, tag="et16d")
                nc.vector.tensor_copy(et16d[: _D, :], ps_et16[:])
                nc.sync.dma_start(et16d[_D :, :], et16d[: _D, :])

                # --- G^T = W @ E16^T + b in PSUM f32 [64, 1024]
                ps_gt = ps_gt_pool.tile([_D, _L], dt.float32, tag="ps_gt")
                for c in range(2):
                    sl = slice(c * 512, (c + 1) * 512)
                    nc.tensor.matmul(
                        ps_gt[:, sl], w2[:], et16d[:, sl], start=True, stop=False
                    )
                    nc.tensor.matmul(
                        ps_gt[:, sl], b2[:], ones2[:], start=False, stop=True
                    )
                # split: gt2 = [Glo (p0-63); Ghi (p64-127)], Ghi staged at p0-63
                gt_hi = gtpool.tile([_D, _L], dt.float16, tag="gthi")
                nc.scalar.activation(gt_hi[:], ps_gt[:], AF.Copy)
                gt2 = gtpool.tile([128, _L], dt.float16, tag="gt2")
                nc.vector.tensor_sub(gt2[: _D, :], ps_gt[:], gt_hi[:])
                nc.sync.dma_start(gt2[_D :, :], gt_hi[:])

                # --- S row-blocks, two t-blocks per 1 MiB output DMA
                for i2 in range(4):
                    exp_sb = bigpool.tile([128, 2 * _L], dt.float32, tag="exp")
                    sums = smallpool.tile([128, 2], dt.float32, tag="sums")
                    recips = smallpool.tile([128, 2], dt.float32, tag="recips")
                    for h in range(2):
                        i = 2 * i2 + h
                        bsl = slice(i * 128, (i + 1) * 128)
                        ps_s = ps_s_pool.tile([128, _L], dt.float32, tag="ps_s")
                        for c in range(2):
                            sl = slice(c * 512, (c + 1) * 512)
                            # ONE matmul per bank: [E16;E16] @ [Glo;Ghi] = E16@G
                            nc.tensor.matmul(
                                ps_s[:, sl], et16d[:, bsl], gt2[:, sl],
                                start=True, stop=True,
                            )
                        # zero the in-block diagonal (j == t)
                        nc.vector.tensor_mul(ps_s[:, bsl], ps_s[:, bsl], anti[:])
                        nc.scalar.activation(
                            exp_sb[:, h * _L : (h + 1) * _L],
                            ps_s[:],
                            AF.Exp,
                            accum_out=sums[:, h : h + 1],
                        )
                    nc.vector.reciprocal(recips[:], sums[:])
                    for h in range(2):
                        nc.vector.tensor_scalar_mul(
                            exp_sb[:, h * _L : (h + 1) * _L],
                            exp_sb[:, h * _L : (h + 1) * _L],
                            recips[:, h : h + 1],
                        )
                    dst = (
                        out_d[2 * i2 * 128 : (2 * i2 + 2) * 128, bb : bb + 1, :]
                        .squeeze(1)
                        .rearrange("(h p) j -> p h j", p=128)
                    )
                    nc.sync.dma_start(dst, exp_sb[:].rearrange("p (h j) -> p h j", h=2))

    nc.compile()
    _compiled_nc = nc
    return nc


def _numpy_fallback(enc, W, b, tl):
    energy = np.einsum("bld,ed->ble", enc, W) + b
    scores = np.einsum("bjd,btd->tbj", energy, enc[:, :tl, :])
    t_idx = np.arange(tl)
    scores[t_idx, :, t_idx] = 0.0
    m = scores.max(axis=-1, keepdims=True)
    e = np.exp(scores - m)
    return (e / e.sum(axis=-1, keepdims=True)).astype(np.float32)


def _run(encoder_outputs, W, b, target_length=1024, **run_kwargs):
    enc = np.ascontiguousarray(np.asarray(encoder_outputs, dtype=np.float32))
    Wn = np.ascontiguousarray(np.asarray(W, dtype=np.float32))
    bn = np.ascontiguousarray(np.asarray(b, dtype=np.float32))
    tl = int(target_length)
    if enc.shape != (_B, _L, _D) or tl != _T:
        return _numpy_fallback(enc, Wn, bn, tl), None

    from concourse.bass_utils import run_bass_kernel_spmd

    nc = _build()
    in_maps = [
        {"enc": enc[i * _BPC : (i + 1) * _BPC], "w": Wn, "bias": bn}
        for i in range(_N_CORES)
    ]
    res = run_bass_kernel_spmd(nc, in_maps, list(range(_N_CORES)), **run_kwargs)
    out = np.concatenate(
        [res.results[i]["out"] for i in range(_N_CORES)], axis=1
    ).astype(np.float32)
    return out, res


def kernel(encoder_outputs, W, b, target_length=1024):
    out, _ = _run(encoder_outputs, W, b, target_length)
    return out


def kernel_profiled(encoder_outputs, W, b, target_length=1024):
    """Run with NTFF tracing; returns (output, BassKernelResults)."""
    return _run(encoder_outputs, W, b, target_length, trace=True)
